# revision 43
# baseline (speedup 1.0000x reference)
"""Trainium2 Bass kernel for nn_Head (single-head causal attention).

Contract: kernel(**inputs) takes FULL inputs (x [8,2048,1024] f32,
Wk/Wq/Wv [64,1024] f32) and returns the FULL output [8,2048,64] f32.
Data-parallel over batch B=8 across the 8 NeuronCores (one batch row per
core); each core runs an identical single-core program.

v5.4 design (descending-chunk streaming; engineered against v4-v5.3 traces):
  * x streams in five t-chunks in REVERSE order: a 128-col micro-chunk
    (t 1920:2048) so projection+attention start after only ~0.5MB of DMA
    (input DMA sustains only ~180-200GB/s aggregate; it paces the head of
    the kernel), then 384/512-col chunks. ST(i) needs kt[:, 128i:2048]
    and qt tile i, so descending i lets attention on chunk c run while
    earlier chunks still DMA. Projections for the NEXT chunk are spread
    through the current chunk's attention (a few matmuls per i) so the PE
    never idles during exp waits and the HAM clock stays at 2.4GHz.
  * Constants/memsets are emitted BEFORE the DMA issues: gpsimd is both
    the SWDGE descriptor generator and the memset engine, and the warm-up
    dummies depend on those memsets.
  * PSUM: stp = 2 rotating [128,1024]f32 slots (tag "st") for ST tiles +
    transient claims; otp = 4 persistent PV banks [128,512]. Bank 0's
    PSUM hosts the kq projections of late chunks before PV first touches
    it at i=3; bank 1 hosts late v projections (first touch i=7).
  * PV bank j accumulates i=4j+3 down to 0. Its first matmul is forced
    full width (below-diagonal pt cols zeroed by gpsimd memset) so
    start=True clears the whole bank.
  * ACT does ONLY Exp until the epilogue (table preloaded in the lead-in;
    evacuation is on DVE; masks + zero-fills on gpsimd).
  * Epilogue: per-bank ACT Copy (one table switch) into bf16 ot_sb, PE
    transposes (bf16 = 1 cyc/row) into freed stp slots, per-partition
    reciprocal [128,1] (cheap; reciprocal scales with FREE size — a
    [1,512] row costs 3.3us!), 2x bf16 per-partition scale, TWO batched
    out DMAs ([T,H] bf16; host casts to f32 — marshaling only).
"""

import sys

if "/opt/trn_rl_repo" not in sys.path:
    sys.path.insert(0, "/opt/trn_rl_repo")

import numpy as np

B = 8
T = 2048
C = 1024
H = 64
P = 128
CB = C // P        # 8 contraction chunks
TJ = T // 512      # 4 column chunks of 512
NT = T // P        # 16 s-tiles
N_CORES = 8

# descending t-chunks: (t0, t1, s-tiles processed)
CHUNKS = [
    (1920, 2048, [15]),
    (1536, 1920, [14, 13, 12]),
    (1024, 1536, [11, 10, 9, 8]),
    (512, 1024, [7, 6, 5, 4]),
    (0, 512, [3, 2, 1, 0]),
]

_NC_CACHE = {}


def _build_nc():
    import concourse.bass as bass
    import concourse.mybir as mybir
    import concourse.tile as tile
    from concourse.bass import ts
    from concourse.masks import make_identity

    fp32 = mybir.dt.float32
    bf16 = mybir.dt.bfloat16
    EXP = mybir.ActivationFunctionType.Exp

    nc = bass.Bass(target_bir_lowering=False, debug=False)
    xt_d = nc.declare_dram_parameter("xt", [C, T], bf16, isOutput=False)
    wkq_d = nc.declare_dram_parameter("wkq", [C, P], bf16, isOutput=False)
    wv_d = nc.declare_dram_parameter("wv", [C, H], bf16, isOutput=False)
    out_d = nc.declare_dram_parameter("out", [T, H], bf16, isOutput=True)

    from contextlib import ExitStack

    with tile.TileContext(nc) as tc, ExitStack() as stk:
        pers = stk.enter_context(tc.tile_pool(name="pers", bufs=1))
        xt_sb = pers.tile([P, CB, T], bf16, tag="xt_sb", name="xt_sb")
        wkq_sb = pers.tile([P, CB, P], bf16, tag="wkq_sb", name="wkq_sb")
        wv_sb = pers.tile([P, CB, H], bf16, tag="wv_sb", name="wv_sb")
        kt_sb = pers.tile([H, T], bf16, tag="kt_sb", name="kt_sb")
        qt_sb = pers.tile([H, T], bf16, tag="qt_sb", name="qt_sb")
        vt_sb = pers.tile([H, T], bf16, tag="vt_sb", name="vt_sb")
        vaug_sb = pers.tile([P, NT, H + 1], bf16, tag="vaug_sb", name="vaug_sb")
        ot_sb = pers.tile([H + 1, T], bf16, tag="ot_sb", name="ot_sb")
        o_sb = pers.tile([P, NT, H], bf16, tag="o_sb", name="o_sb")
        rec_sb = pers.tile([P, NT], fp32, tag="rec_sb", name="rec_sb")
        o2_sb = pers.tile([H, T], bf16, tag="o2_sb", name="o2_sb")
        identf = pers.tile([H + 1, H + 1], bf16, tag="identf", name="identf")
        identb = pers.tile([H, H], bf16, tag="identb", name="identb")
        identp = pers.tile([P, P], bf16, tag="identp", name="identp")
        dummy_sb = pers.tile([P, 512], bf16, tag="dummy_sb", name="dummy_sb")
        tl_sb = pers.tile([1, 8], fp32, tag="tl_sb", name="tl_sb")

        # ---- constants FIRST (gpsimd also runs SWDGE descriptor generation
        # for its DMA queue; the warm-up dummies depend on these memsets) ----
        make_identity(nc, identb[:])
        make_identity(nc, identf[:])
        make_identity(nc, identp[:])
        nc.gpsimd.memset(dummy_sb[:], 0.0)
        nc.gpsimd.memset(tl_sb[:], 0.0)
        nc.any.memset(vaug_sb[:, :, H], 1.0)
        nc.scalar.activation(tl_sb[:], tl_sb[:], EXP)

        # ---- input DMAs: weights + the 128-col micro-chunk first on the
        # two HWDGE queues, then the remaining t-ranges striped across
        # sync/scalar/gpsimd, newest-needed first ----
        wkq_r = wkq_d.rearrange("(o p) m -> p o m", p=P)
        xt_r = xt_d.rearrange("(o p) m -> p o m", p=P)
        nc.sync.dma_start(wkq_sb[:, 0:4, :], wkq_r[:, 0:4, :])
        nc.sync.dma_start(xt_sb[:, 0:4, 1920:2048], xt_r[:, 0:4, 1920:2048])
        nc.scalar.dma_start(wkq_sb[:, 4:8, :], wkq_r[:, 4:8, :])
        nc.scalar.dma_start(xt_sb[:, 4:8, 1920:2048], xt_r[:, 4:8, 1920:2048])
        # wv rides the gpsimd SWDGE queue: its x stripes aren't needed until
        # chunk 1, so wv lands early without delaying the kq weights
        nc.gpsimd.dma_start(wv_sb[:], wv_d.rearrange("(o p) m -> p o m", p=P))
        engs = [nc.sync, nc.scalar, nc.gpsimd]
        stripes = [(0, 3), (3, 6), (6, 8)]
        for t0, t1, _ in CHUNKS[1:]:
            for q, (a, b) in enumerate(stripes):
                engs[q].dma_start(xt_sb[:, a:b, t0:t1], xt_r[:, a:b, t0:t1])

        # ---- HAM warm-up: dummy matmuls bridge the DMA lead-in so real
        # matmuls run at 2.4GHz from the start ----
        with tc.tile_pool(name="warm", bufs=1, space="PSUM") as wp:
            wps = wp.tile([H, P], fp32, tag="w", name="warm")
            for _ in range(26):
                nc.tensor.matmul(
                    wps, identb[:], dummy_sb[0:H, 0:P], start=True, stop=True
                )

        with (
            tc.tile_pool(name="stp", bufs=2, space="PSUM") as stp,
            tc.tile_pool(name="otp", bufs=4, space="PSUM") as otp,
            tc.tile_pool(name="ptp", bufs=6) as ptp,
        ):
            # full [128,512] handles: rows 0:65 are the PV accumulators;
            # bank 0 / bank 1 host late-chunk kq / v projections first
            ot_full = [otp.tile([P, 512], fp32, tag="ot", name=f"otf{j}") for j in range(TJ)]
            ot_ps = [otf[0 : H + 1, :] for otf in ot_full]

            def emit_st(i):
                j0 = i // 4
                pts = {}
                for jj2 in range(i // 8, 2):
                    st = stp.tile([P, 1024], fp32, tag="st", name=f"st{i}_{jj2}")
                    pt = ptp.tile([P, 1024], bf16, tag="pt", name=f"pt{i}_{jj2}")
                    estart = None
                    for hh in range(2):
                        j = 2 * jj2 + hh
                        if j < j0:
                            continue
                        o = max(0, 128 * i - 512 * j)
                        lo = 512 * hh + o
                        nc.tensor.matmul(
                            st[:, lo : 512 * (hh + 1)], qt_sb[:, ts(i, P)],
                            kt_sb[:, 512 * j + o : 512 * (j + 1)],
                            start=True, stop=True,
                        )
                        if estart is None:
                            estart = lo
                    nc.scalar.activation(pt[:, estart:1024], st[:, estart:1024], EXP)
                    if jj2 == i // 8:
                        # causal mask of the diagonal 128x128 block:
                        # keep pt[s, t] where t - s >= 0, else 0
                        dlo = 128 * (i % 8)
                        nc.gpsimd.affine_select(
                            out=pt[:, dlo : dlo + P],
                            in_=pt[:, dlo : dlo + P],
                            pattern=[[1, P]],
                            compare_op=mybir.AluOpType.is_ge,
                            fill=0.0,
                            base=0,
                            channel_multiplier=-1,
                        )
                    if i % 4 == 3 and jj2 == i // 8:
                        # this i opens PV bank j0: zero the below-diagonal
                        # cols of the piece so the bank's first PV matmul can
                        # be full width (start=True then clears the whole bank)
                        zlo = 512 * (j0 % 2)
                        nc.gpsimd.memset(pt[:, zlo : zlo + 384], 0.0)
                    pts[jj2] = pt
                return pts

            def emit_pv(i, pts):
                j0 = i // 4
                for j in range(j0, TJ):
                    o = 0 if i == 4 * j + 3 else max(0, 128 * i - 512 * j)
                    pt = pts[j // 2]
                    lo = 512 * (j % 2) + o
                    nc.tensor.matmul(
                        ot_ps[j][:, o:512], vaug_sb[:, i, :],
                        pt[:, lo : 512 * (j % 2) + 512],
                        start=(i == 4 * j + 3), stop=(i == 0),
                    )

            def proj_mms(target, w_sb, t0, t1, cbs):
                for cb in cbs:
                    nc.tensor.matmul(
                        target[:, 0 : t1 - t0], w_sb[:, cb, :], xt_sb[:, cb, t0:t1],
                        start=(cb == 0), stop=(cb == CB - 1),
                    )

            # chunk 0 (micro) and chunk 1 projections use stp slots in the
            # lead-in; later chunks are staged in ot banks 0/1
            kq_t = {}
            v_t = {}
            kq_t[0] = stp.tile([P, 128], fp32, tag="st", name="kqc0")
            v_t[0] = stp.tile([H, 128], fp32, tag="st", name="vc0")
            proj_mms(kq_t[0], wkq_sb, 1920, 2048, range(CB))
            proj_mms(v_t[0][0:H, :], wv_sb, 1920, 2048, range(CB))

            # spread-projection schedule: list per chunk index of
            # (kind, chunk_idx, cb-list) emitted per i-iteration
            def spread_plan(ci):
                if ci == 1:  # project chunk 2 (3 iterations available)
                    return [[("kq", 2, range(0, 4)), ("v", 2, range(0, 2))],
                            [("kq", 2, range(4, 8)), ("v", 2, range(2, 4))],
                            [("v", 2, range(4, 8))]]
                if ci == 2:  # project chunk 3
                    return [[("kq", 3, range(0, 4))], [("kq", 3, range(4, 8))],
                            [("v", 3, range(0, 4))], [("v", 3, range(4, 8))]]
                if ci == 3:  # project chunk 4's kq only (v needs an stp slot)
                    return [[("kq", 4, range(0, 2))], [("kq", 4, range(2, 4))],
                            [("kq", 4, range(4, 6))], [("kq", 4, range(6, 8))]]
                return [[] for _ in range(4)]

            prev = None
            for ci, (t0, t1, tiles) in enumerate(CHUNKS):
                if ci == 1:
                    kq_t[1] = stp.tile([P, 384], fp32, tag="st", name="kqc1")
                    proj_mms(kq_t[1], wkq_sb, 1536, 1920, range(CB))
                    v_t[1] = stp.tile([H, 384], fp32, tag="st", name="vc1")
                    proj_mms(v_t[1][0:H, :], wv_sb, 1536, 1920, range(CB))
                if ci == 4:
                    v_t[4] = stp.tile([H, 512], fp32, tag="st", name="vc4")
                    proj_mms(v_t[4][0:H, :], wv_sb, 0, 512, range(CB))
                kq = kq_t.get(ci, ot_full[0])
                vv = v_t[ci][0:H, :] if ci in v_t else ot_full[1][0:H, :]
                w = t1 - t0
                # evacuate kq first and emit this chunk's FIRST ST before any
                # v-side work: the first exp starts while v evac/transposes
                # (which only PV needs, one lag step later) proceed
                nc.vector.tensor_copy(kt_sb[:, t0:t1], kq[0:H, 0:w])
                nc.vector.tensor_copy(qt_sb[:, t0:t1], kq[H:P, 0:w])
                pts = emit_st(tiles[0])
                if prev is not None:
                    emit_pv(prev[0], prev[1])
                prev = (tiles[0], pts)
                nc.vector.tensor_copy(vt_sb[:, t0:t1], vv[:, 0:w])
                # v natural layout: PE transposes + one wide DVE copy
                nblk = w // P
                vg = stp.tile([P, nblk, H], bf16, tag="st", name=f"vg{ci}")
                for bkl in range(nblk):
                    nc.tensor.transpose(
                        vg[:, bkl, :], vt_sb[:, t0 + P * bkl : t0 + P * (bkl + 1)],
                        identb[:],
                    )
                i0 = t0 // P
                nc.vector.tensor_copy(vaug_sb[:, i0 : i0 + nblk, 0:H], vg)
                if ci == 0:
                    # second HAM bridge: the PE would idle ~4us here waiting
                    # for chunk 1's DMA and re-chill to 1.2GHz just as the
                    # attention stream begins
                    dmt = stp.tile([H, 512], fp32, tag="st", name="dmt")
                    for _ in range(14):
                        nc.tensor.matmul(
                            dmt[:, 0:256], identb[:], dummy_sb[0:H, 0:256],
                            start=True, stop=True,
                        )
                # remaining tiles, descending i; PV lags ST by one i; next
                # chunk's proj matmuls fill the exp-wait gaps
                plan = spread_plan(ci)
                for n, i in enumerate(tiles[1:] + [None]):
                    if i is not None:
                        pts = emit_st(i)
                        emit_pv(prev[0], prev[1])
                    if n < len(plan):
                        for kind, pc, cbs in plan[n]:
                            tgt = ot_full[0] if kind == "kq" else ot_full[1][0:H, :]
                            proj_mms(tgt, wkq_sb if kind == "kq" else wv_sb,
                                     CHUNKS[pc][0], CHUNKS[pc][1], cbs)
                    if i is not None:
                        prev = (i, pts)
            emit_pv(prev[0], prev[1])

            # ---- epilogue: banks close together at PV(0). Bank copies on
            # ACT (one Exp->Copy table switch) in parallel with DVE; per
            # 128-tile: bf16 PE transpose into a freed stp slot, cheap
            # per-partition reciprocal, 2x bf16 scale; two batched DMAs ----
            COPY = mybir.ActivationFunctionType.Copy
            out_r = out_d.rearrange("(i p) d -> p i d", p=P)
            for j in range(TJ):
                # bank evacuations split across ACT and DVE so they drain
                # in parallel with the recip/scale chain
                if j % 2 == 0:
                    nc.scalar.activation(ot_sb[:, ts(j, 512)], ot_ps[j], COPY)
                else:
                    nc.vector.tensor_copy(ot_sb[:, ts(j, 512)], ot_ps[j])
                for ii in range(4 * j, 4 * j + 4):
                    ops = stp.tile([P, H + 1], bf16, tag="st", name=f"or{ii}")
                    nc.tensor.transpose(ops, ot_sb[:, ts(ii, P)], identf[:])
                    nc.vector.reciprocal(rec_sb[:, ii : ii + 1], ops[:, H : H + 1])
                    nc.vector.tensor_scalar_mul(
                        o_sb[:, ii, :], ops[:, 0:H], rec_sb[:, ii : ii + 1]
                    )
                # ship each bank as soon as its 4 tiles are normalized
                eng = nc.sync if j % 2 == 0 else nc.scalar
                eng.dma_start(
                    out_r[:, 4 * j : 4 * j + 4, :], o_sb[:, 4 * j : 4 * j + 4, :]
                )

    return nc


def _split_multiwaits(nc):
    """Walrus codegen only supports one sync-wait command per instruction;
    hoist extra waits onto NoOps inserted just before (same engine queue,
    identical semantics since engines execute their queue in order)."""
    import concourse.mybir as mybir

    n = 0
    for fn in nc.m.functions:
        for block in fn.blocks:
            new_insts = []
            for inst in block.instructions:
                si = inst.sync_info
                if si is not None and si.on_wait and len(si.on_wait) > 1:
                    waits = list(si.on_wait)
                    for w in waits[:-1]:
                        n += 1
                        new_insts.append(
                            mybir.InstNoOp(
                                name=f"WH-{n}", engine=inst.engine, ins=[], outs=[],
                                sync_info=mybir.SyncInfo(on_wait=[w], on_update=[]),
                            )
                        )
                    si.on_wait = waits[-1:]
                new_insts.append(inst)
            block.instructions = new_insts
    return nc


def _get_nc():
    if "nc" not in _NC_CACHE:
        _NC_CACHE["nc"] = _split_multiwaits(_build_nc())
    return _NC_CACHE["nc"]


def _make_in_maps(x, Wk, Wq, Wv):
    import ml_dtypes

    bf16 = ml_dtypes.bfloat16
    scale = 1.0 / np.sqrt(np.float32(C))
    wkq = np.ascontiguousarray(
        np.concatenate([Wk * scale, Wq], axis=0).T.astype(bf16)
    )  # [C, 128]
    wv = np.ascontiguousarray(Wv.T.astype(bf16))  # [C, 64]
    in_maps = []
    for b in range(B):
        xt = np.ascontiguousarray(x[b].T.astype(bf16))  # [C, T]
        in_maps.append({"xt": xt, "wkq": wkq, "wv": wv})
    return in_maps


def run(x, Wk, Wq, Wv, trace=False):
    from concourse.bass_utils import run_bass_kernel_spmd

    nc = _get_nc()
    in_maps = _make_in_maps(x, Wk, Wq, Wv)
    res = run_bass_kernel_spmd(nc, in_maps, core_ids=list(range(N_CORES)), trace=trace)
    # device output is [T, H] bf16; f32 cast is host-side marshaling
    out = np.stack(
        [np.asarray(res.results[b]["out"]).astype(np.float32) for b in range(B)],
        axis=0,
    )
    return np.ascontiguousarray(out), res


def kernel(x, Wk, Wq, Wv):
    out, _ = run(x, Wk, Wq, Wv, trace=False)
    return out


# revision 46
# speedup vs baseline: 1.0360x; 1.0360x over previous
"""Trainium2 Bass kernel for nn_Head (single-head causal attention).

Contract: kernel(**inputs) takes FULL inputs (x [8,2048,1024] f32,
Wk/Wq/Wv [64,1024] f32) and returns the FULL output [8,2048,64] f32.
Data-parallel over batch B=8 across the 8 NeuronCores (one batch row per
core); each core runs an identical single-core program.

v5.4 design (descending-chunk streaming; engineered against v4-v5.3 traces):
  * x streams in five t-chunks in REVERSE order: a 128-col micro-chunk
    (t 1920:2048) so projection+attention start after only ~0.5MB of DMA
    (input DMA sustains only ~180-200GB/s aggregate; it paces the head of
    the kernel), then 384/512-col chunks. ST(i) needs kt[:, 128i:2048]
    and qt tile i, so descending i lets attention on chunk c run while
    earlier chunks still DMA. Projections for the NEXT chunk are spread
    through the current chunk's attention (a few matmuls per i) so the PE
    never idles during exp waits and the HAM clock stays at 2.4GHz.
  * Constants/memsets are emitted BEFORE the DMA issues: gpsimd is both
    the SWDGE descriptor generator and the memset engine, and the warm-up
    dummies depend on those memsets.
  * PSUM: stp = 2 rotating [128,1024]f32 slots (tag "st") for ST tiles +
    transient claims; otp = 4 persistent PV banks [128,512]. Bank 0's
    PSUM hosts the kq projections of late chunks before PV first touches
    it at i=3; bank 1 hosts late v projections (first touch i=7).
  * PV bank j accumulates i=4j+3 down to 0. Its first matmul is forced
    full width (below-diagonal pt cols zeroed by gpsimd memset) so
    start=True clears the whole bank.
  * ACT does ONLY Exp until the epilogue (table preloaded in the lead-in;
    evacuation is on DVE; masks + zero-fills on gpsimd).
  * Epilogue: per-bank ACT Copy (one table switch) into bf16 ot_sb, PE
    transposes (bf16 = 1 cyc/row) into freed stp slots, per-partition
    reciprocal [128,1] (cheap; reciprocal scales with FREE size — a
    [1,512] row costs 3.3us!), 2x bf16 per-partition scale, TWO batched
    out DMAs ([T,H] bf16; host casts to f32 — marshaling only).
"""

import sys

if "/opt/trn_rl_repo" not in sys.path:
    sys.path.insert(0, "/opt/trn_rl_repo")

import numpy as np

B = 8
T = 2048
C = 1024
H = 64
P = 128
CB = C // P        # 8 contraction chunks
TJ = T // 512      # 4 column chunks of 512
NT = T // P        # 16 s-tiles
N_CORES = 8

# descending t-chunks: (t0, t1, s-tiles processed)
CHUNKS = [
    (1920, 2048, [15]),
    (1536, 1920, [14, 13, 12]),
    (1024, 1536, [11, 10, 9, 8]),
    (512, 1024, [7, 6, 5, 4]),
    (0, 512, [3, 2, 1, 0]),
]

_NC_CACHE = {}


def _build_nc():
    import concourse.bass as bass
    import concourse.mybir as mybir
    import concourse.tile as tile
    from concourse.bass import ts
    from concourse.masks import make_identity

    fp32 = mybir.dt.float32
    bf16 = mybir.dt.bfloat16
    EXP = mybir.ActivationFunctionType.Exp

    nc = bass.Bass(target_bir_lowering=False, debug=False)
    xt_d = nc.declare_dram_parameter("xt", [C, T], bf16, isOutput=False)
    wkq_d = nc.declare_dram_parameter("wkq", [C, P], bf16, isOutput=False)
    wv_d = nc.declare_dram_parameter("wv", [C, H], bf16, isOutput=False)
    out_d = nc.declare_dram_parameter("out", [T, H], bf16, isOutput=True)

    from contextlib import ExitStack

    with tile.TileContext(nc) as tc, ExitStack() as stk:
        pers = stk.enter_context(tc.tile_pool(name="pers", bufs=1))
        xt_sb = pers.tile([P, CB, T], bf16, tag="xt_sb", name="xt_sb")
        wkq_sb = pers.tile([P, CB, P], bf16, tag="wkq_sb", name="wkq_sb")
        wv_sb = pers.tile([P, CB, H], bf16, tag="wv_sb", name="wv_sb")
        kt_sb = pers.tile([H, T], bf16, tag="kt_sb", name="kt_sb")
        qt_sb = pers.tile([H, T], bf16, tag="qt_sb", name="qt_sb")
        vt_sb = pers.tile([H, T], bf16, tag="vt_sb", name="vt_sb")
        vaug_sb = pers.tile([P, NT, H + 1], bf16, tag="vaug_sb", name="vaug_sb")
        ot_sb = pers.tile([H + 1, T], bf16, tag="ot_sb", name="ot_sb")
        o_sb = pers.tile([P, NT, H], bf16, tag="o_sb", name="o_sb")
        rec_sb = pers.tile([P, NT], fp32, tag="rec_sb", name="rec_sb")
        identf = pers.tile([H + 1, H + 1], bf16, tag="identf", name="identf")
        identb = pers.tile([H, H], bf16, tag="identb", name="identb")
        dummy_sb = pers.tile([P, 512], bf16, tag="dummy_sb", name="dummy_sb")
        tl_sb = pers.tile([1, 8], fp32, tag="tl_sb", name="tl_sb")

        # ---- constants FIRST (gpsimd also runs SWDGE descriptor generation
        # for its DMA queue; the warm-up dummies depend on these memsets) ----
        make_identity(nc, identb[:])
        make_identity(nc, identf[:])
        nc.gpsimd.memset(dummy_sb[:], 0.0)
        nc.gpsimd.memset(tl_sb[:], 0.0)
        nc.any.memset(vaug_sb[:, :, H], 1.0)
        nc.scalar.activation(tl_sb[:], tl_sb[:], EXP)

        # ---- input DMAs: weights + the 128-col micro-chunk first on the
        # two HWDGE queues, then the remaining t-ranges striped across
        # sync/scalar/gpsimd, newest-needed first ----
        wkq_r = wkq_d.rearrange("(o p) m -> p o m", p=P)
        xt_r = xt_d.rearrange("(o p) m -> p o m", p=P)
        nc.sync.dma_start(wkq_sb[:, 0:4, :], wkq_r[:, 0:4, :])
        nc.sync.dma_start(xt_sb[:, 0:4, 1920:2048], xt_r[:, 0:4, 1920:2048])
        nc.scalar.dma_start(wkq_sb[:, 4:8, :], wkq_r[:, 4:8, :])
        nc.scalar.dma_start(xt_sb[:, 4:8, 1920:2048], xt_r[:, 4:8, 1920:2048])
        # wv rides the gpsimd SWDGE queue: its x stripes aren't needed until
        # chunk 1, so wv lands early without delaying the kq weights
        nc.gpsimd.dma_start(wv_sb[:], wv_d.rearrange("(o p) m -> p o m", p=P))
        engs = [nc.sync, nc.scalar, nc.gpsimd]
        stripes = [(0, 3), (3, 6), (6, 8)]
        for t0, t1, _ in CHUNKS[1:]:
            for q, (a, b) in enumerate(stripes):
                engs[q].dma_start(xt_sb[:, a:b, t0:t1], xt_r[:, a:b, t0:t1])

        # ---- HAM warm-up: dummy matmuls bridge the DMA lead-in so real
        # matmuls run at 2.4GHz from the start ----
        with tc.tile_pool(name="warm", bufs=1, space="PSUM") as wp:
            wps = wp.tile([H, P], fp32, tag="w", name="warm")
            for _ in range(26):
                nc.tensor.matmul(
                    wps, identb[:], dummy_sb[0:H, 0:P], start=True, stop=True
                )

        with (
            tc.tile_pool(name="stp", bufs=2, space="PSUM") as stp,
            tc.tile_pool(name="otp", bufs=4, space="PSUM") as otp,
            tc.tile_pool(name="ptp", bufs=6) as ptp,
        ):
            # full [128,512] handles: rows 0:65 are the PV accumulators;
            # bank 0 / bank 1 host late-chunk kq / v projections first
            ot_full = [otp.tile([P, 512], fp32, tag="ot", name=f"otf{j}") for j in range(TJ)]
            ot_ps = [otf[0 : H + 1, :] for otf in ot_full]

            def emit_st(i):
                j0 = i // 4
                pts = {}
                for jj2 in range(i // 8, 2):
                    st = stp.tile([P, 1024], fp32, tag="st", name=f"st{i}_{jj2}")
                    pt = ptp.tile([P, 1024], bf16, tag="pt", name=f"pt{i}_{jj2}")
                    estart = None
                    for hh in range(2):
                        j = 2 * jj2 + hh
                        if j < j0:
                            continue
                        o = max(0, 128 * i - 512 * j)
                        lo = 512 * hh + o
                        nc.tensor.matmul(
                            st[:, lo : 512 * (hh + 1)], qt_sb[:, ts(i, P)],
                            kt_sb[:, 512 * j + o : 512 * (j + 1)],
                            start=True, stop=True,
                        )
                        if estart is None:
                            estart = lo
                    nc.scalar.activation(pt[:, estart:1024], st[:, estart:1024], EXP)
                    if jj2 == i // 8:
                        # causal mask of the diagonal 128x128 block:
                        # keep pt[s, t] where t - s >= 0, else 0
                        dlo = 128 * (i % 8)
                        nc.gpsimd.affine_select(
                            out=pt[:, dlo : dlo + P],
                            in_=pt[:, dlo : dlo + P],
                            pattern=[[1, P]],
                            compare_op=mybir.AluOpType.is_ge,
                            fill=0.0,
                            base=0,
                            channel_multiplier=-1,
                        )
                    if i % 4 == 3 and jj2 == i // 8:
                        # this i opens PV bank j0: zero the below-diagonal
                        # cols of the piece so the bank's first PV matmul can
                        # be full width (start=True then clears the whole bank)
                        zlo = 512 * (j0 % 2)
                        nc.gpsimd.memset(pt[:, zlo : zlo + 384], 0.0)
                    pts[jj2] = pt
                return pts

            def emit_pv(i, pts):
                j0 = i // 4
                for j in range(j0, TJ):
                    o = 0 if i == 4 * j + 3 else max(0, 128 * i - 512 * j)
                    pt = pts[j // 2]
                    lo = 512 * (j % 2) + o
                    nc.tensor.matmul(
                        ot_ps[j][:, o:512], vaug_sb[:, i, :],
                        pt[:, lo : 512 * (j % 2) + 512],
                        start=(i == 4 * j + 3), stop=(i == 0),
                    )

            def proj_mms(target, w_sb, t0, t1, cbs):
                for cb in cbs:
                    nc.tensor.matmul(
                        target[:, 0 : t1 - t0], w_sb[:, cb, :], xt_sb[:, cb, t0:t1],
                        start=(cb == 0), stop=(cb == CB - 1),
                    )

            # chunk 0 (micro) and chunk 1 projections use stp slots in the
            # lead-in; later chunks are staged in ot banks 0/1
            kq_t = {}
            v_t = {}
            kq_t[0] = stp.tile([P, 128], fp32, tag="st", name="kqc0")
            v_t[0] = stp.tile([H, 128], fp32, tag="st", name="vc0")
            proj_mms(kq_t[0], wkq_sb, 1920, 2048, range(CB))
            proj_mms(v_t[0][0:H, :], wv_sb, 1920, 2048, range(CB))

            # spread-projection schedule: list per chunk index of
            # (kind, chunk_idx, cb-list) emitted per i-iteration
            def spread_plan(ci):
                if ci == 1:  # project chunk 2 (3 iterations available)
                    return [[("kq", 2, range(0, 4)), ("v", 2, range(0, 2))],
                            [("kq", 2, range(4, 8)), ("v", 2, range(2, 4))],
                            [("v", 2, range(4, 8))]]
                if ci == 2:  # project chunk 3
                    return [[("kq", 3, range(0, 4))], [("kq", 3, range(4, 8))],
                            [("v", 3, range(0, 4))], [("v", 3, range(4, 8))]]
                if ci == 3:  # project chunk 4's kq only (v needs an stp slot)
                    return [[("kq", 4, range(0, 2))], [("kq", 4, range(2, 4))],
                            [("kq", 4, range(4, 6))], [("kq", 4, range(6, 8))]]
                return [[] for _ in range(4)]

            prev = None
            for ci, (t0, t1, tiles) in enumerate(CHUNKS):
                if ci == 1:
                    kq_t[1] = stp.tile([P, 384], fp32, tag="st", name="kqc1")
                    proj_mms(kq_t[1], wkq_sb, 1536, 1920, range(CB))
                    v_t[1] = stp.tile([H, 384], fp32, tag="st", name="vc1")
                    proj_mms(v_t[1][0:H, :], wv_sb, 1536, 1920, range(CB))
                if ci == 4:
                    v_t[4] = stp.tile([H, 512], fp32, tag="st", name="vc4")
                    proj_mms(v_t[4][0:H, :], wv_sb, 0, 512, range(CB))
                kq = kq_t.get(ci, ot_full[0])
                vv = v_t[ci][0:H, :] if ci in v_t else ot_full[1][0:H, :]
                w = t1 - t0
                # evacuate kq first and emit this chunk's FIRST ST before any
                # v-side work: the first exp starts while v evac/transposes
                # (which only PV needs, one lag step later) proceed
                nc.vector.tensor_copy(kt_sb[:, t0:t1], kq[0:H, 0:w])
                nc.vector.tensor_copy(qt_sb[:, t0:t1], kq[H:P, 0:w])
                pts = emit_st(tiles[0])
                if prev is not None:
                    emit_pv(prev[0], prev[1])
                prev = (tiles[0], pts)
                nc.vector.tensor_copy(vt_sb[:, t0:t1], vv[:, 0:w])
                # v natural layout: PE transposes + one wide DVE copy
                nblk = w // P
                vg = stp.tile([P, nblk, H], bf16, tag="st", name=f"vg{ci}")
                for bkl in range(nblk):
                    nc.tensor.transpose(
                        vg[:, bkl, :], vt_sb[:, t0 + P * bkl : t0 + P * (bkl + 1)],
                        identb[:],
                    )
                i0 = t0 // P
                nc.vector.tensor_copy(vaug_sb[:, i0 : i0 + nblk, 0:H], vg)
                # remaining tiles, descending i; PV lags ST by one i; next
                # chunk's proj matmuls fill the exp-wait gaps
                plan = spread_plan(ci)
                for n, i in enumerate(tiles[1:] + [None]):
                    if i is not None:
                        pts = emit_st(i)
                        emit_pv(prev[0], prev[1])
                    if n < len(plan):
                        for kind, pc, cbs in plan[n]:
                            tgt = ot_full[0] if kind == "kq" else ot_full[1][0:H, :]
                            proj_mms(tgt, wkq_sb if kind == "kq" else wv_sb,
                                     CHUNKS[pc][0], CHUNKS[pc][1], cbs)
                    if i is not None:
                        prev = (i, pts)
            emit_pv(prev[0], prev[1])

            # ---- epilogue: banks close together at PV(0). Bank copies on
            # ACT (one Exp->Copy table switch) in parallel with DVE; per
            # 128-tile: bf16 PE transpose into a freed stp slot, cheap
            # per-partition reciprocal, 2x bf16 scale; two batched DMAs ----
            COPY = mybir.ActivationFunctionType.Copy
            out_r = out_d.rearrange("(i p) d -> p i d", p=P)
            for j in range(TJ):
                # bank evacuations split across ACT and DVE so they drain
                # in parallel with the recip/scale chain
                if j % 2 == 0:
                    nc.scalar.activation(ot_sb[:, ts(j, 512)], ot_ps[j], COPY)
                else:
                    nc.vector.tensor_copy(ot_sb[:, ts(j, 512)], ot_ps[j])
                for ii in range(4 * j, 4 * j + 4):
                    ops = stp.tile([P, H + 1], bf16, tag="st", name=f"or{ii}")
                    nc.tensor.transpose(ops, ot_sb[:, ts(ii, P)], identf[:])
                    nc.vector.reciprocal(rec_sb[:, ii : ii + 1], ops[:, H : H + 1])
                    nc.vector.tensor_scalar_mul(
                        o_sb[:, ii, :], ops[:, 0:H], rec_sb[:, ii : ii + 1]
                    )
                # ship each bank as soon as its 4 tiles are normalized
                eng = nc.sync if j % 2 == 0 else nc.scalar
                eng.dma_start(
                    out_r[:, 4 * j : 4 * j + 4, :], o_sb[:, 4 * j : 4 * j + 4, :]
                )

    return nc


def _split_multiwaits(nc):
    """Walrus codegen only supports one sync-wait command per instruction;
    hoist extra waits onto NoOps inserted just before (same engine queue,
    identical semantics since engines execute their queue in order)."""
    import concourse.mybir as mybir

    n = 0
    for fn in nc.m.functions:
        for block in fn.blocks:
            new_insts = []
            for inst in block.instructions:
                si = inst.sync_info
                if si is not None and si.on_wait and len(si.on_wait) > 1:
                    waits = list(si.on_wait)
                    for w in waits[:-1]:
                        n += 1
                        new_insts.append(
                            mybir.InstNoOp(
                                name=f"WH-{n}", engine=inst.engine, ins=[], outs=[],
                                sync_info=mybir.SyncInfo(on_wait=[w], on_update=[]),
                            )
                        )
                    si.on_wait = waits[-1:]
                new_insts.append(inst)
            block.instructions = new_insts
    return nc


def _get_nc():
    if "nc" not in _NC_CACHE:
        _NC_CACHE["nc"] = _split_multiwaits(_build_nc())
    return _NC_CACHE["nc"]


def _make_in_maps(x, Wk, Wq, Wv):
    import ml_dtypes

    bf16 = ml_dtypes.bfloat16
    scale = 1.0 / np.sqrt(np.float32(C))
    wkq = np.ascontiguousarray(
        np.concatenate([Wk * scale, Wq], axis=0).T.astype(bf16)
    )  # [C, 128]
    wv = np.ascontiguousarray(Wv.T.astype(bf16))  # [C, 64]
    in_maps = []
    for b in range(B):
        xt = np.ascontiguousarray(x[b].T.astype(bf16))  # [C, T]
        in_maps.append({"xt": xt, "wkq": wkq, "wv": wv})
    return in_maps


def run(x, Wk, Wq, Wv, trace=False):
    from concourse.bass_utils import run_bass_kernel_spmd

    nc = _get_nc()
    in_maps = _make_in_maps(x, Wk, Wq, Wv)
    res = run_bass_kernel_spmd(nc, in_maps, core_ids=list(range(N_CORES)), trace=trace)
    # device output is [T, H] bf16; f32 cast is host-side marshaling
    out = np.stack(
        [np.asarray(res.results[b]["out"]).astype(np.float32) for b in range(B)],
        axis=0,
    )
    return np.ascontiguousarray(out), res


def kernel(x, Wk, Wq, Wv):
    out, _ = run(x, Wk, Wq, Wv, trace=False)
    return out
